# revision 1
# baseline (speedup 1.0000x reference)
"""JacobianDetLoss Trainium2 kernel, v2 (bf16 + PE shift-matmul).

Full inputs: disp (2,3,192,192,192) f32, mask (2,1,192,192,192) f32.
Output: scalar f32 loss = sum(relu(-det)*mask) / (sum(mask)+1e-6).

Sharding: X axis split into 8 slabs of 24 det-planes (+1 halo plane). Each
core processes 4 rounds (b in {0,1} x y-window in {0..96, 95..191}); the
y-derivative is computed on the TensorEngine with a +-1 shift-difference
stationary matrix, evacuated PSUM->SBUF by the scalar engine (with +1 bias
folded in for the a11 entry), and the determinant/cofactor algebra runs on
the vector engine in packed bf16. All DMAs use 96-partition transfers (97-
partition DMAs hit a pathological slow path) with the halo row loaded
separately. Duplicate-counted mask rows/planes are host-zeroed so the
numerator counts every voxel exactly once; the denominator is an exact
host-side sum. Each core returns 8 per-(round,half) partial-sum vectors;
the host reduces and divides.
"""
import sys

sys.path.insert(0, "/opt/trn_rl_repo")

import numpy as np
import ml_dtypes

BF16 = ml_dtypes.bfloat16
EPS = 1e-6
S = 192
B = 2
NX = 25  # disp planes per core (24 det planes + 1 halo)
NCORES = 8
NY = 97  # y rows per tile (96 + 1 halo)


def _build_program(repeat=1):
    from contextlib import ExitStack

    import concourse.bass as bass
    from concourse import mybir

    bf = mybir.dt.bfloat16
    f32 = mybir.dt.float32
    op = mybir.AluOpType

    nc = bass.Bass()
    d_ext = nc.declare_dram_parameter("d", [B, 2, NY, NX, 3, S], bf, isOutput=False)
    m_ext = nc.declare_dram_parameter("m", [B, 2, 96, NX, S], bf, isOutput=False)
    s_ext = nc.declare_dram_parameter("s", [NY, 96], bf, isOutput=False)
    out_ext = nc.declare_dram_parameter("partial", [96, 8], f32, isOutput=True)

    es = ExitStack()

    def sb(name, shape, dt=bf):
        h = nc.sbuf_tensor(name, shape, dt)
        if hasattr(h, "__enter__"):
            h = es.enter_context(h)
        return h

    def ps(name, shape):
        h = nc.psum_tensor(name, shape, mybir.dt.float32)
        if hasattr(h, "__enter__"):
            h = es.enter_context(h)
        return h

    D = [sb(f"D{k}", [NY, NX, 3, S]) for k in range(2)]
    M = [sb(f"M{k}", [96, NX, S]) for k in range(2)]
    Dy = [sb(f"Dy{k}", [96, 24, 3, S]) for k in range(2)]
    Dz = sb("Dz", [96, 24, 3, S])
    Dx = sb("Dx", [96, 12, 3, S])
    T1 = sb("T1", [96, 12, 3, S])
    T2 = sb("T2", [96, 12, 3, S])
    Ssb = sb("Ssb", [NY, 96])
    rnum = sb("rnum", [96, 8], f32)
    P = [ps(f"P{k}", [128, 4, 512]) for k in range(2)]

    rounds = [(b, h) for b in range(B) for h in range(2)] * repeat
    CSLOT = [1, 2, 0]  # channel for Dy slot: slot0=dy1(a11,+1), 1=dy2, 2=dy0
    NFILL = 9  # psum fills per round (3 xquads x 3 cslots)

    with (
        nc.Block() as block,
        nc.semaphore("d_sem") as d_sem,
        nc.semaphore("m_sem") as m_sem,
        nc.semaphore("s_sem") as s_sem,
        nc.semaphore("mm_sem") as mm_sem,
        nc.semaphore("cp_sem") as cp_sem,
        nc.semaphore("dyfree") as dyfree,     # DVE done with Dy buffer (per round)
        nc.semaphore("dfree_v") as dfree_v,
        nc.semaphore("mfree") as mfree,
        nc.semaphore("done_sem") as done_sem,
        nc.semaphore("out_sem") as out_sem,
    ):

        @block.sync
        def _(sync: bass.BassEngine):
            sync.dma_start(out=Ssb[0:96, :], in_=s_ext[0:96, :]).then_inc(s_sem, 16)
            sync.dma_start(out=Ssb[96:97, :], in_=s_ext[96:97, :]).then_inc(s_sem, 16)
            for i, (b, h) in enumerate(rounds):
                if i >= 2:
                    # all D readers of round i-2 retired: PE via its copies
                    # (cp implies matmuls), Pool via dzp, vector via dfree_v
                    sync.wait_ge(cp_sem, NFILL * (i - 1))
                    sync.wait_ge(dfree_v, i - 1)
                sync.dma_start(
                    out=D[i % 2][0:96, :, :, :], in_=d_ext[b, h, 0:96]
                ).then_inc(d_sem, 16)
                sync.dma_start(
                    out=D[i % 2][96:97, :, :, :], in_=d_ext[b, h, 96:97]
                ).then_inc(d_sem, 16)
                if i >= 2:
                    sync.wait_ge(mfree, i - 1)
                sync.dma_start(
                    out=M[i % 2][:, :, :], in_=m_ext[b, h]
                ).then_inc(m_sem, 16)
            sync.wait_ge(done_sem, 1)
            sync.dma_start(out=out_ext[:, :], in_=rnum[:, :]).then_inc(out_sem, 16)
            sync.wait_ge(out_sem, 16)

        @block.tensor
        def _(pe: bass.BassEngine):
            pe.wait_ge(s_sem, 32)
            fill = 0  # global fill counter
            for i, (b, h) in enumerate(rounds):
                pe.wait_ge(d_sem, 32 * (i + 1))
                if i >= 2:
                    # Dy[i%2] still in use by vector until round i-2 ends
                    pe.wait_ge(dyfree, i - 1)
                for q in range(3):  # x-quads: 4 xpairs = 8 planes each
                    for cs in range(3):
                        c = CSLOT[cs]
                        if fill >= 2:
                            pe.wait_ge(cp_sem, fill - 1)
                        pt = P[fill % 2]
                        for j in range(4):
                            xp = 4 * q + j  # xpair index 0..11
                            pe.matmul(
                                out=pt[0:96, j, 0:384],
                                lhsT=Ssb[0:NY, 0:96],
                                rhs=D[i % 2][0:NY, 2 * xp : 2 * xp + 2, c, :],
                                start=True,
                                stop=True,
                            ).then_inc(mm_sem, 1)
                        fill += 1

        @block.scalar
        def _(act: bass.BassEngine):
            for i, (b, h) in enumerate(rounds):
                for f in range(NFILL):
                    gf = NFILL * i + f
                    q, cs = f // 3, f % 3
                    act.wait_ge(mm_sem, 4 * (gf + 1))
                    act.activation(
                        out=Dy[i % 2][0:96, 8 * q : 8 * q + 8, cs, :],
                        in_=P[gf % 2][0:96, 0:4, 0:384],
                        func=mybir.ActivationFunctionType.Copy,
                        bias=1.0 if cs == 0 else 0.0,
                        scale=1.0,
                    ).then_inc(cp_sem, 1)

        @block.vector
        def _(v: bass.BassEngine):
            v.memset(Dz[:, :, :, 191], 0.0)
            for i, (b, h) in enumerate(rounds):
                Dc = D[i % 2]
                Dyc = Dy[i % 2]
                v.wait_ge(d_sem, 32 * (i + 1))
                # Dz slot0 = dz0, slot1 = dz1 (channels 0,1)
                v.tensor_tensor(
                    out=Dz[0:96, :, 0:2, 0:191],
                    in0=Dc[0:96, 0:24, 0:2, 1:192],
                    in1=Dc[0:96, 0:24, 0:2, 0:191],
                    op=op.subtract,
                )
                # Dz slot2 = dz2 + 1 (a22)
                v.scalar_tensor_tensor(
                    out=Dz[0:96, :, 2, 0:191],
                    in0=Dc[0:96, 0:24, 2, 1:192],
                    scalar=1.0,
                    op0=op.add,
                    in1=Dc[0:96, 0:24, 2, 0:191],
                    op1=op.subtract,
                )
                for hx in range(2):
                    k = 2 * i + hx  # global half index
                    xs = 12 * hx
                    # Dx = D[x+1] - D[x] for all 3 channels of this x-half
                    dxi = v.tensor_tensor(
                        out=Dx[0:96, :, :, :],
                        in0=Dc[0:96, xs + 1 : xs + 13, :, :],
                        in1=Dc[0:96, xs : xs + 12, :, :],
                        op=op.subtract,
                    )
                    if hx == 1:
                        dxi.then_inc(dfree_v, 1)
                    # a00 = dx0 + 1
                    v.tensor_scalar_add(Dx[0:96, :, 0, :], Dx[0:96, :, 0, :], 1.0)
                    if hx == 0:
                        v.wait_ge(cp_sem, NFILL * i + 6)
                    else:
                        v.wait_ge(cp_sem, NFILL * (i + 1))
                    # T1 = [a11*a22, a12*a20, a10*a21]
                    v.tensor_tensor(
                        out=T1[0:96, :, 0, :],
                        in0=Dyc[0:96, xs : xs + 12, 0, :],
                        in1=Dz[0:96, xs : xs + 12, 2, :],
                        op=op.mult,
                    )
                    v.tensor_tensor(
                        out=T1[0:96, :, 1:3, :],
                        in0=Dyc[0:96, xs : xs + 12, 1:3, :],
                        in1=Dz[0:96, xs : xs + 12, 0:2, :],
                        op=op.mult,
                    )
                    # T2 = [a12*a21, a10*a22, a11*a20]
                    v.tensor_tensor(
                        out=T2[0:96, :, 0:2, :],
                        in0=Dyc[0:96, xs : xs + 12, 1:3, :],
                        in1=Dz[0:96, xs : xs + 12, 1:3, :],
                        op=op.mult,
                    )
                    v.tensor_tensor(
                        out=T2[0:96, :, 2, :],
                        in0=Dyc[0:96, xs : xs + 12, 0, :],
                        in1=Dz[0:96, xs : xs + 12, 0, :],
                        op=op.mult,
                    )
                    # M1 = T1 - T2 (cofactors C00, C01, C02), in place into T1
                    v.tensor_tensor(
                        out=T1[0:96, :, :, :],
                        in0=T1[0:96, :, :, :],
                        in1=T2[0:96, :, :, :],
                        op=op.subtract,
                    )
                    # DxM = Dx' * M1, written into this half's Dy columns
                    # (their last readers, T1/T2 of this half, are done)
                    v.tensor_tensor(
                        out=Dyc[0:96, xs : xs + 12, :, :],
                        in0=Dx[0:96, :, :, :],
                        in1=T1[0:96, :, :, :],
                        op=op.mult,
                    )
                    # det = DxM0 + DxM1 + DxM2, ending in T1 slot0
                    v.tensor_tensor(
                        out=T1[0:96, :, 1, :],
                        in0=Dyc[0:96, xs : xs + 12, 0, :],
                        in1=Dyc[0:96, xs : xs + 12, 1, :],
                        op=op.add,
                    )
                    det3 = v.tensor_tensor(
                        out=T1[0:96, :, 0, :],
                        in0=T1[0:96, :, 1, :],
                        in1=Dyc[0:96, xs : xs + 12, 2, :],
                        op=op.add,
                    )
                    if hx == 1:
                        det3.then_inc(dyfree, 1)
                    # pm = min(det,0)*mask, accumulated into slot k
                    v.wait_ge(m_sem, 16 * (i + 1))
                    pmi = v.scalar_tensor_tensor(
                        out=Dx[0:96, :, 0, 0:191],
                        in0=T1[0:96, :, 0, 0:191],
                        scalar=0.0,
                        op0=op.min,
                        in1=M[i % 2][0:96, xs : xs + 12, 0:191],
                        op1=op.mult,
                        accum_out=rnum[0:96, k % 8 : k % 8 + 1],
                    )
                    if hx == 1:
                        # data-visible completion marker on the producer
                        if i == len(rounds) - 1:
                            pmi.then_inc(done_sem, 1)
                        else:
                            pmi.then_inc(mfree, 1)

    es.close()
    return nc


_NC_CACHE = None
_LAST_RES = None


def _prep_inputs(disp: np.ndarray, mask: np.ndarray):
    disp = np.asarray(disp, dtype=np.float32)
    mask = np.asarray(mask, dtype=np.float32)

    d = disp[:, ::-1]  # channel reversal (torch [2,1,0] order)
    # layout [b, y, x, c, z], bf16
    dt = np.ascontiguousarray(d.transpose(0, 3, 2, 1, 4)).astype(BF16)
    mt = np.ascontiguousarray(mask[:, 0].transpose(0, 2, 1, 3)).astype(BF16)

    ys = [0, 95]  # y-window starts (windows of 97 rows)
    smat = np.zeros((NY, 96), dtype=BF16)
    idx = np.arange(96)
    smat[idx + 1, idx] = 1.0
    smat[idx, idx] = -1.0

    in_maps = []
    for c in range(NCORES):
        xs = 24 * c if c < 7 else 167
        dslab = np.empty((B, 2, NY, NX, 3, S), dtype=BF16)
        mslab = np.empty((B, 2, 96, NX, S), dtype=BF16)
        for b in range(B):
            for h in range(2):
                y0 = ys[h]
                dslab[b, h] = dt[b, y0 : y0 + NY, xs : xs + NX]
                mslab[b, h] = mt[b, y0 : y0 + 96, xs : xs + NX]
                # zero the duplicate-counted y row (h1 row 0 = h0's last row)
                if h == 1:
                    mslab[b, h, 0] = 0.0
                # zero the duplicate-counted x plane (pm uses planes 0..23)
                if c == 7:
                    mslab[b, h, :, 0] = 0.0
        in_maps.append({"d": dslab, "m": mslab, "s": smat})
    return in_maps


def kernel(disp: np.ndarray, mask: np.ndarray) -> np.ndarray:
    global _NC_CACHE, _LAST_RES
    from concourse.bass_utils import run_bass_kernel_spmd

    in_maps = _prep_inputs(disp, mask)

    if _NC_CACHE is None:
        _NC_CACHE = _build_program()
    res = run_bass_kernel_spmd(_NC_CACHE, in_maps, core_ids=list(range(NCORES)))
    _LAST_RES = res
    num = 0.0
    for r in res.results:
        num -= np.asarray(r["partial"], dtype=np.float64).sum()
    den = float(np.asarray(mask, dtype=np.float32).sum(dtype=np.float64))
    return np.float32(num / (den + EPS))

